# revision 56
# baseline (speedup 1.0000x reference)
"""BinaryTreeRNN Trainium2 kernel — 8-core data-parallel, fp16 pipeline.

Contract: kernel(**inputs) takes FULL unsharded inputs (x [4M,16] f32 plus tiny
tree params) and returns the FULL [4M] f32 output.

Design (per core, N_core = 500k samples, padded to 501760 = 560 blocks x 896):
  * Host folds tree params (float64):  softmax(om) -> per-node (A,P,R,phi,B);
    the combine  o = A*s + R*sin(s+phi) + P*l*r + B  is refactored as
      o = HL*HR + R*sin(theta) + const,   HL = c_hl*(P*l+A), HR = c_hr*(r+A/P)
    (factored quadratic absorbs the linear A*s term).  Stored values carry
    affine maps  true = S*stored + T  folded into the next level's constants;
    per-level power-of-2 scales keep everything in fp16 range.
  * PE: per block one fp16 matmul  out[p,c] = sum_k xt[k,p]*wp[k,c] producing
    12 funcs x 7 slots = 84 cols: HL3/HR3 (4 nodes, L2-pair-permuted) and
    sc3 = (s3+phi3)/2pi.  Bias via two constant rows (112=hi,113=lo).
  * Tree on DVE/ACT in fp16.  Sin range reduction via write-rounding magic:
    ACT copies sc3+1536 PSUM->fp16 (the fp16 write rounds to integer+1536),
    then one scalar_tensor_tensor recovers f = round(sc)-sc from the
    full-precision PSUM sc3; L2 rounds in fp16 (ts_add +1536 / -1536), L1 in
    fp32 (phases exceed fp16's +/-512 magic range).  Per-node constants ride
    broadcast-AP (stride-0) tensor_tensor operands; hr_sc is forced to 1 by
    scale choice so HR needs only a bias add.
  * Emission is software-pipelined: group g's wide-phase chunks interleave
    with group g+1's DMA/matmul/PSUM-evacuation so the in-order engines
    always have ready work; first/last groups are small (ramp/tail).
"""

import os
import sys

for _p in ("/opt/trn_rl_repo", "/root/.axon_site/_ro/trn_rl_repo"):
    if os.path.isdir(_p) and _p not in sys.path:
        sys.path.append(_p)

import numpy as np

N_FULL = 4_000_000
V = 16
N_CORES = 8
N_CORE = N_FULL // N_CORES          # 500_000
SLOTS = 7
BLK = 128 * SLOTS                   # 896
N_BLOCKS = 560
N_PAD = N_BLOCKS * BLK              # 501_760
B = 16                              # blocks per supertile
N_ST = N_BLOCKS // B                # 35
GROUP = 11                          # supertiles per group

TWO_PI = 2.0 * np.pi
M16 = 1536.0                        # fp16 round-to-int magic
PERM = [0, 2, 1, 3]                 # L3 node order: L2 pairs contiguous

F16 = np.float16
F32 = np.float32
F64 = np.float64


def _softmax64(om):
    e = np.exp(om.astype(F64) - om.astype(F64).max(-1, keepdims=True))
    return e / e.sum(-1, keepdims=True)


def _lvl(w, b, om):
    sm = _softmax64(om)
    w64 = w.astype(F64)
    A = w64 * sm[:, 0]
    S = w64 * sm[:, 1]
    C = w64 * sm[:, 2]
    P = w64 * sm[:, 3]
    return dict(A=A, B=b.astype(F64), P=P, R=np.hypot(S, C),
                phi=np.arctan2(C, S))


def _pow2(v):
    return float(2.0 ** np.round(np.log2(max(abs(float(v)), 1e-30))))


def _fold(leaf_w, leaf_b, w1, b1, om1, w2, b2, om2, w3, b3, om3, x_sample):
    """float64 constant folding -> (wp fp16 [128,84], consts dict)."""
    L3 = _lvl(w3, b3, om3)
    L2 = _lvl(w2, b2, om2)
    L1 = _lvl(w1, b1, om1)
    lw = leaf_w.astype(F64)
    lb = leaf_b.astype(F64)
    h = (x_sample.astype(F64) @ lw.T + lb).T      # [8, M]

    def calib(vals, target=2.0):
        return _pow2(target / (np.sqrt((vals ** 2).mean()) + 1e-30))

    # ---- L3 ----
    n3 = []
    o3t = []
    for n in range(4):
        A, P, R, phi, Bc = (L3[k][n] for k in ("A", "P", "R", "phi", "B"))
        l, r = h[2 * n], h[2 * n + 1]
        c_hl = calib(P * l + A)
        c_hr = calib(r + A / P)
        o3t.append(A * (l + r) + R * np.sin(l + r + phi) + P * l * r + Bc)
        n3.append(dict(A=A, P=P, R=R, phi=phi, B=Bc, c_hl=c_hl, c_hr=c_hr,
                       wl=lw[2 * n], wr=lw[2 * n + 1], bl=lb[2 * n],
                       br=lb[2 * n + 1]))
    # shared scale fixed to exactly 1/2pi: stored o3 sums ARE the L2
    # phase in periods, so the L2 sin path needs no rescale op
    cc = 1.0 / TWO_PI
    for d in n3:
        d["c_hr"] *= cc / (d["c_hl"] * d["c_hr"])
        d["c"] = cc
        d["S"] = 1.0 / cc
        d["T"] = d["B"] - d["A"] ** 2 / d["P"]

    # ---- L2 ----  (c_hr forced so hr_sc == 1: HR2 = o3r_stored + hr_b,
    # no multiply needed; c_hl carries all the pow2 balance freedom)
    cc3 = cc
    c_hl_raw = []
    o2t = []
    for m in range(2):
        A, P, R, phi, Bc = (L2[k][m] for k in ("A", "P", "R", "phi", "B"))
        l, r = o3t[2 * m], o3t[2 * m + 1]
        c_hl_raw.append(calib(P * l + A))
        o2t.append(A * (l + r) + R * np.sin(l + r + phi) + P * l * r + Bc)
    cc2 = 1.0 / TWO_PI                  # L2 stored scale also exactly 1/2pi
    n2 = []
    for m in range(2):
        A, P, R, phi, Bc = (L2[k][m] for k in ("A", "P", "R", "phi", "B"))
        cl, cr = n3[2 * m], n3[2 * m + 1]
        c_hr = cc3                      # -> hr_sc = cr["S"]*c_hr = 1
        c_hl = cc2 / cc3
        th_b = cl["T"] + cr["T"] + phi
        n2.append(dict(
            A=A, P=P, R=R, phi=phi, B=Bc, c_hl=c_hl, c_hr=c_hr,
            hl_sc=P * cl["S"] * c_hl, hl_b=(A + P * cl["T"]) * c_hl,
            hr_sc=1.0, hr_b=(cr["T"] + A / P) * c_hr,
            sc_sc=cl["S"] / TWO_PI,
            dfrac=(th_b / TWO_PI) - np.round(th_b / TWO_PI),
            c=cc2, S=1.0 / cc2, T=Bc - A ** 2 / P,
        ))

    # ---- L1 ----  (same hr_sc == 1 construction)
    A, P, R, phi, Bc = (L1[k][0] for k in ("A", "P", "R", "phi", "B"))
    cl, cr = n2
    l, r = o2t
    c_hr = cc2
    c_hl = calib(P * l + A)
    th_b = cl["T"] + cr["T"] + phi
    n1 = dict(
        A=A, P=P, R=R, phi=phi, B=Bc, c_hl=c_hl, c_hr=c_hr,
        hl_sc=P * cl["S"] * c_hl, hl_b=(A + P * cl["T"]) * c_hl,
        hr_sc=1.0, hr_b=(cr["T"] + A / P) * c_hr,
        sc_sc=cl["S"] / TWO_PI,
        dfrac=(th_b / TWO_PI) - np.round(th_b / TWO_PI),
        c=c_hl * c_hr,
    )
    n1["S"] = 1.0 / n1["c"]
    n1["T"] = Bc - A ** 2 / P
    # sanity: HR tensors are o_stored + hr_b; biases must stay in fp16 range
    assert abs(n1["hr_b"]) < 3e4 and all(abs(d["hr_b"]) < 3e4 for d in n2)
    # stored o2 = o2_true/2pi must stay inside fp16 range incl. tails
    assert max(np.abs(v).max() for v in o2t) * 2.0 / TWO_PI < 6e4, \
        "stored o2 exceeds fp16 range with S2 = 2pi"

    # ---- PE weight matrix [128, 84]: col 7j+a ----
    wp = np.zeros((128, 84), F64)
    for j, n in enumerate(PERM):
        d = n3[n]
        cols = [
            (j, d["wl"] * d["P"] * d["c_hl"],
             (d["P"] * d["bl"] + d["A"]) * d["c_hl"]),
            (4 + j, d["wr"] * d["c_hr"], (d["br"] + d["A"] / d["P"]) * d["c_hr"]),
            (8 + j, (d["wl"] + d["wr"]) / TWO_PI,
             (d["bl"] + d["br"] + d["phi"]) / TWO_PI),
        ]
        for jj, wv, bias in cols:
            for a_ in range(SLOTS):
                wp[16 * a_:16 * a_ + 16, 7 * jj + a_] = wv
                bh = np.float16(bias)
                wp[112, 7 * jj + a_] = bh
                wp[113, 7 * jj + a_] = np.float16(bias - float(bh))
    wp16 = wp.astype(F16)

    # packed broadcast-constant columns [119]:
    #  0:28  r3bc   28:42 r2bc   42:56 d2bc(dfrac)  56:70 hs2  70:84 hb2
    #  84:98 rs2   98:112 rb2   112:119 r1bc
    cst = np.zeros(119, F64)
    for j, n in enumerate(PERM):
        cst[7 * j:7 * j + 7] = n3[n]["R"] * n3[n]["c"]
    for m in range(2):
        s = slice(28 + 7 * m, 35 + 7 * m)
        cst[s.start:s.stop] = n2[m]["R"] * n2[m]["c"]
        cst[s.start + 14:s.stop + 14] = n2[m]["dfrac"]
        cst[s.start + 28:s.stop + 28] = n2[m]["hl_sc"]
        cst[s.start + 42:s.stop + 42] = n2[m]["hl_b"]
        cst[s.start + 56:s.stop + 56] = n2[m]["hr_sc"]
        cst[s.start + 70:s.stop + 70] = n2[m]["hr_b"]
    cst[112:119] = n1["R"] * n1["c"]
    cst16 = np.broadcast_to(cst.astype(F16), (128, 119)).copy()

    # fp16 magic rounding at L3/L2 requires |theta|/2pi well below 512
    mx3 = max(np.abs(h[2 * n] + h[2 * n + 1] + n3[n]["phi"]).max()
              for n in range(4)) / TWO_PI
    mx2 = max(np.abs(o3t[2 * m] + o3t[2 * m + 1] + n2[m]["phi"]).max()
              for m in range(2)) / TWO_PI
    # 2.5x extrapolation subsample-max -> full-N max; beyond 512 a tail
    # sample gets a bounded sign-flipped sin (negligible in L2 norm),
    # beyond ~2048 sin output explodes -> hard error.
    assert mx3 * 2.5 < 500 and mx2 * 2.5 < 2000, \
        f"fp16 sin-magic range exceeded: sc3 max {mx3:.1f}, sc2 max {mx2:.1f}"

    consts = dict(L2=n2, L1=n1, S=n1["S"], T=n1["T"])
    return wp16, cst16, consts


def _pack_x(x_shard, n_st=N_ST, b_blocks=B):
    """[n,16] f32 -> fp16 [n_st, 112, b_blocks*128] stationary rows."""
    npad = n_st * b_blocks * BLK
    xs = np.empty((npad, V), F32)
    xs[:len(x_shard)] = x_shard
    xs[len(x_shard):] = 1.0
    a = xs.reshape(n_st, b_blocks, 128, SLOTS, V)      # [st, b, p, a, v]
    xt = a.transpose(0, 3, 4, 1, 2).reshape(n_st, 112, b_blocks * 128)
    return np.ascontiguousarray(xt, dtype=F16)


_PROGRAM_CACHE = {}


def _build_program(n_st=N_ST, b_blocks=B, group=GROUP):
    import json
    key = (n_st, b_blocks, group,
           json.dumps(_build_program.consts, sort_keys=True, default=str))
    if key in _PROGRAM_CACHE:
        return _PROGRAM_CACHE[key]

    import concourse.bass as bass
    import concourse.tile as tile
    from concourse import bacc, mybir
    from contextlib import ExitStack

    f32 = mybir.dt.float32
    f16 = mybir.dt.float16
    Sin = mybir.ActivationFunctionType.Sin
    Ident = mybir.ActivationFunctionType.Identity
    sub = mybir.AluOpType.subtract
    mult = mybir.AluOpType.mult
    addop = mybir.AluOpType.add

    C = _build_program.consts
    n2, n1 = C["L2"], C["L1"]
    M32 = float(1.5 * 2.0 ** 23)

    nc = bacc.Bacc("TRN2", target_bir_lowering=False, debug=False,
                   num_devices=N_CORES)
    xh_d = nc.dram_tensor("xh", [n_st, 112, b_blocks * 128], f16,
                          kind="ExternalInput")
    wp_d = nc.dram_tensor("wp", [128, 84], f16, kind="ExternalInput")
    cst_d = nc.dram_tensor("cst", [128, 119], f16, kind="ExternalInput")
    out_d = nc.dram_tensor("out", [n_st, 128, b_blocks, SLOTS], f16,
                           kind="ExternalOutput")

    def reg_const(v):
        v = float(F32(v))
        if (f32, v) not in nc.const_aps.aps:
            t = nc.alloc_sbuf_tensor(
                f"constx-{len(nc.const_aps.aps)}", [128, 1], f32)
            nc.gpsimd.memset(t.ap(), v)
            nc.const_aps.aps[(f32, v)] = t.ap()

    reg_const(0.0)
    reg_const(M16)
    for d in n2:
        reg_const(TWO_PI * d["dfrac"])
    warm = nc.alloc_sbuf_tensor("sinwarm", [128, 1], f32)
    nc.gpsimd.memset(warm.ap(), 0.0)
    nc.all_engine_barrier()
    # warm up the Sin spline table set as the first ACT op: the ~2.7us
    # ACT_TABLE_LOAD overlaps the initial DMAs/matmuls instead of sitting
    # on the first group's critical path
    nc.scalar.activation(warm.ap(), warm.ap(), Sin, bias=0.0, scale=1.0)

    with tile.TileContext(nc) as tc:
        with ExitStack() as ctx:
            const_pool = ctx.enter_context(tc.tile_pool(name="const", bufs=1))
            xpool = ctx.enter_context(tc.tile_pool(name="x", bufs=1))
            ppool = ctx.enter_context(
                tc.tile_pool(name="ps", bufs=2, space=bass.MemorySpace.PSUM))
            g2pool = ctx.enter_context(tc.tile_pool(name="g2", bufs=2))
            g1pool = ctx.enter_context(tc.tile_pool(name="g1", bufs=1))

            wp = const_pool.tile([128, 84], f16)
            nc.sync.dma_start(wp[:], wp_d[:])
            cst = const_pool.tile([128, 119], f16)
            nc.sync.dma_start(cst[:], cst_d[:])

            def bc(lo, hi, q):
                return cst[:, lo:hi].unsqueeze(1).broadcast_to(
                    (128, q, hi - lo))

            xts = []
            for i in range(2):
                t = xpool.tile([128, b_blocks * 128], f16, name=f"xt{i}",
                               tag=f"xt{i}")
                # start partition must be a multiple of 32; rows 96:112 are
                # rewritten by every x DMA, rows 112:128 stay constant 1.0
                nc.gpsimd.memset(t[96:128, :], 1.0)
                xts.append(t)

            def alloc_group():
                """Seg-phase-filled tiles (double-buffered across groups)."""
                tt = {}
                for nm in ("g3", "k3", "hl3", "f3"):
                    t = g2pool.tile([128, group * b_blocks * 28], f16,
                                    name=nm, tag=nm)
                    tt[nm] = (t, t[:].rearrange("p (q c) -> p q c", c=28))
                return tt

            def emit_seg(tiles, st):
                xt = xts[st % 2]
                nc.sync.dma_start(xt[0:112, :], xh_d[st])
                ps = ppool.tile([128, b_blocks * 128], f32)
                for b in range(b_blocks):
                    nc.tensor.matmul(ps[:, 128 * b:128 * b + 84],
                                     xt[:, 128 * b:128 * b + 128],
                                     wp[:], start=True, stop=True)
                psv = ps[:].rearrange("p (b c) -> p b c", c=128)
                seg = emit_seg.idx
                emit_seg.idx += 1
                ssl = slice(seg * b_blocks, (seg + 1) * b_blocks)
                hl3v = tiles["hl3"][1]
                k3v = tiles["k3"][1]
                # PSUM evacuation (one PSUM operand per op); k3/f3 first so
                # DVE unblocks after a single ACT copy:
                #   k3  <- sc3 + M16 (fp16 write rounds to integer+M16)
                #   f3  <- (k3 - M16) - sc3[psum]
                #   hl3 <- HL cols;  g3 <- hl3 * HR[psum]
                nc.scalar.activation(k3v[:, ssl, :], psv[:, :, 56:84],
                                     Ident, bias=M16, scale=1.0)
                nc.vector.scalar_tensor_tensor(
                    tiles["f3"][1][:, ssl, :], k3v[:, ssl, :], M16,
                    psv[:, :, 56:84], sub, sub)
                nc.scalar.activation(hl3v[:, ssl, :], psv[:, :, 0:28],
                                     Ident, bias=0.0, scale=1.0)
                nc.vector.tensor_tensor(tiles["g3"][1][:, ssl, :],
                                        hl3v[:, ssl, :],
                                        psv[:, :, 28:56], mult)

            def wide_chunks(tiles, glen, st0):
                """Yield the wide-phase as a list of closures (chunks)."""
                q = glen * b_blocks
                qsl = slice(0, q)
                qf3, qf2, qf1 = q * 28, q * 14, q * 7
                g3 = tiles["g3"][0]
                f3 = tiles["f3"][0]

                def gt(cols, nm, dt=f16):
                    t = g1pool.tile([128, group * b_blocks * cols], dt,
                                    name=nm, tag=nm)
                    return t, t[:].rearrange("p (q c) -> p q c", c=cols)

                st_ = {}

                def c_t3u3():
                    t3, t3v = gt(28, "t3")
                    nc.scalar.activation(t3[:, 0:qf3], f3[:, 0:qf3], Sin,
                                         bias=0.0, scale=float(F32(-TWO_PI)))
                    u3, u3v = gt(28, "u3")
                    nc.vector.tensor_tensor(u3v[:, qsl, :], t3v[:, qsl, :],
                                            bc(0, 28, q), mult)
                    st_["u3"] = u3

                def c_o3():
                    o3, o3v = gt(28, "o3")
                    nc.vector.tensor_tensor(o3[:, 0:qf3], g3[:, 0:qf3],
                                            st_["u3"][:, 0:qf3], addop)
                    st_["o3"] = (o3, o3v)

                def c_s2():
                    # stored o3 is scaled exactly 1/2pi: s2 IS the phase in
                    # periods; round directly (per-node dfrac in the magic)
                    o3, o3v = st_["o3"]
                    st_["l2v"] = o3v[:, qsl, 0:14]
                    st_["r2v"] = o3v[:, qsl, 14:28]
                    s2, s2v = gt(14, "s2")
                    nc.vector.tensor_tensor(s2v[:, qsl, :], st_["l2v"],
                                            st_["r2v"], addop)
                    st_["s2"] = (s2, s2v)

                def c_k2():
                    s2, s2v = st_["s2"]
                    k2, k2v = gt(14, "k2")
                    for m in range(2):
                        d = n2[m]
                        nc.vector.tensor_scalar_add(
                            k2v[:, qsl, 7 * m:7 * m + 7],
                            s2v[:, qsl, 7 * m:7 * m + 7],
                            float(F32(M16 + d["dfrac"])))
                    nc.vector.tensor_scalar_sub(k2[:, 0:qf2], k2[:, 0:qf2],
                                                M16)
                    st_["k2"] = k2

                def c_f2():
                    s2, s2v = st_["s2"]
                    f2, f2v = gt(14, "f2")
                    nc.vector.tensor_tensor(f2[:, 0:qf2],
                                            st_["k2"][:, 0:qf2],
                                            s2[:, 0:qf2], sub)
                    t2, t2v = gt(14, "t2")
                    for m in range(2):
                        d = n2[m]
                        nc.scalar.activation(
                            t2v[:, qsl, 7 * m:7 * m + 7],
                            f2v[:, qsl, 7 * m:7 * m + 7], Sin,
                            bias=float(F32(TWO_PI * d["dfrac"])),
                            scale=float(F32(-TWO_PI)))
                    u2, u2v = gt(14, "u2")
                    nc.vector.tensor_tensor(u2v[:, qsl, :], t2v[:, qsl, :],
                                            bc(28, 42, q), mult)
                    st_["u2"] = u2

                def c_HL2():
                    HL2, HL2v = gt(14, "HL2")
                    nc.vector.tensor_tensor(HL2v[:, qsl, :], st_["l2v"],
                                            bc(56, 70, q), mult)
                    for m in range(2):
                        d = n2[m]
                        sl7 = (slice(None), qsl, slice(7 * m, 7 * m + 7))
                        nc.vector.tensor_scalar_add(HL2v[sl7], HL2v[sl7],
                                                    float(F32(d["hl_b"])))
                    st_["HL2"] = HL2

                def c_HR2():
                    HR2, HR2v = gt(14, "HR2")
                    for m in range(2):
                        d = n2[m]
                        sl7 = (slice(None), qsl, slice(7 * m, 7 * m + 7))
                        nc.vector.tensor_scalar_add(
                            HR2v[sl7], st_["r2v"][:, :, 7 * m:7 * m + 7],
                            float(F32(d["hr_b"])))
                    g2t, _ = gt(14, "g2")
                    nc.vector.tensor_tensor(g2t[:, 0:qf2],
                                            st_["HL2"][:, 0:qf2],
                                            HR2[:, 0:qf2], mult)
                    st_["g2"] = g2t

                def c_o2():
                    o2, o2v = gt(14, "o2")
                    nc.vector.tensor_tensor(o2[:, 0:qf2], st_["g2"][:, 0:qf2],
                                            st_["u2"][:, 0:qf2], addop)
                    st_["o2"] = (o2, o2v)

                def c_s1():
                    o2, o2v = st_["o2"]
                    st_["l1v"] = o2v[:, qsl, 0:7]
                    st_["r1v"] = o2v[:, qsl, 7:14]
                    d = n1
                    s1, s1v = gt(7, "s1")
                    nc.vector.tensor_tensor(s1v[:, qsl, :], st_["l1v"],
                                            st_["r1v"], addop)
                    # sc_sc == 1 (stored scale is 1/2pi): the f32 convert
                    # and dfrac shift merge into one ts_add
                    sc1, _ = gt(7, "sc1", dt=f32)
                    nc.vector.tensor_scalar_add(sc1[:, 0:qf1], s1[:, 0:qf1],
                                                float(F32(d["dfrac"])))
                    st_["sc1"] = sc1

                def c_f1():
                    d = n1
                    sc1 = st_["sc1"]
                    k1, _ = gt(7, "k1", dt=f32)
                    nc.vector.tensor_scalar_add(k1[:, 0:qf1], sc1[:, 0:qf1],
                                                M32)
                    f1, _ = gt(7, "f1")
                    nc.vector.scalar_tensor_tensor(
                        f1[:, 0:qf1], k1[:, 0:qf1], M32, sc1[:, 0:qf1],
                        sub, sub)
                    t1, t1v = gt(7, "t1")
                    nc.scalar.activation(t1[:, 0:qf1], f1[:, 0:qf1], Sin,
                                         bias=0.0, scale=float(F32(-TWO_PI)))
                    st_["t1"] = (t1, t1v)

                def c_g1():
                    d = n1
                    HL1, HL1v = gt(7, "HL1")
                    nc.vector.tensor_scalar_mul(HL1v[:, qsl, :], st_["l1v"],
                                                float(F32(d["hl_sc"])))
                    nc.vector.tensor_scalar_add(HL1[:, 0:qf1], HL1[:, 0:qf1],
                                                float(F32(d["hl_b"])))
                    HR1, HR1v = gt(7, "HR1")
                    nc.vector.tensor_scalar_add(HR1v[:, qsl, :], st_["r1v"],
                                                float(F32(d["hr_b"])))
                    g1t, _ = gt(7, "g1")
                    nc.vector.tensor_tensor(g1t[:, 0:qf1], HL1[:, 0:qf1],
                                            HR1[:, 0:qf1], mult)
                    st_["g1"] = g1t

                def c_yo():
                    # L1 has a single node: R is one scalar -> fused stt
                    t1, t1v = st_["t1"]
                    yo, _ = gt(7, "yo")
                    nc.vector.scalar_tensor_tensor(
                        yo[:, 0:qf1], t1[:, 0:qf1],
                        float(F32(n1["R"] * n1["c"])), st_["g1"][:, 0:qf1],
                        mult, addop)
                    dst = out_d[st0:st0 + glen].transpose([1, 0, 2, 3])
                    yo4 = yo[:, 0:qf1].rearrange("p (g b a) -> p g b a",
                                                 g=glen, a=SLOTS)
                    nc.sync.dma_start(dst, yo4)

                return [c_t3u3, c_o3, c_s2, c_k2, c_f2, c_HL2, c_HR2,
                        c_o2, c_s1, c_f1, c_g1, c_yo]

            # group sizes: small first group shortens the pipeline fill
            # ramp; small last group shortens the un-overlapped tail
            glens = []
            rem = n_st
            if n_st > group + 5:
                glens.append(2)
                rem -= 2
                tail = 3 if rem % group == 1 else (rem % group) or 3
                while rem > tail:
                    glens.append(min(group, rem - tail))
                    rem -= glens[-1]
                glens.append(rem)
                rem = 0
            while rem > 0:
                glens.append(min(group, rem))
                rem -= glens[-1]

            # software pipeline: interleave group g's wide chunks with
            # group g+1's seg ops so the (in-order) engines always have
            # ready work queued.
            prev = None
            st0 = 0
            for glen in glens:
                tiles = alloc_group()
                emit_seg.idx = 0
                segs = list(range(st0, st0 + glen))
                if prev is None:
                    for st in segs:
                        emit_seg(tiles, st)
                else:
                    # chunk-first: the first chunk (t3/u3) depends only on
                    # the previous group and keeps ACT/DVE busy while the
                    # new group's DMAs/matmuls start.
                    chunks = wide_chunks(*prev)
                    si = ci = 0
                    while ci < len(chunks) or si < len(segs):
                        if ci < len(chunks):
                            chunks[ci]()
                            ci += 1
                        if si < len(segs) and (ci * len(segs)
                                               >= si * len(chunks)):
                            emit_seg(tiles, segs[si])
                            si += 1
                prev = (tiles, glen, st0)
                st0 += glen
            for c in wide_chunks(*prev):
                c()

    nc.compile()
    _PROGRAM_CACHE[key] = nc
    return nc


def kernel(x, leaf_w, leaf_b, w1, b1, om1, w2, b2, om2, w3, b3, om3):
    from concourse.bass_interp import get_hw_module
    from concourse.bass_utils import run_bass_kernel_spmd

    x = np.ascontiguousarray(x, dtype=F32)
    wp, cst, consts = _fold(
        leaf_w, leaf_b, w1, b1, om1, w2, b2, om2, w3, b3, om3,
        x[:: max(1, N_FULL // 4096)][:4096])
    _build_program.consts = consts
    nc = _build_program()

    in_maps = []
    for c in range(N_CORES):
        xh = _pack_x(x[c * N_CORE:(c + 1) * N_CORE])
        in_maps.append({"xh": xh, "wp": wp, "cst": cst})

    kw = {}
    if os.environ.get("KERNEL_TRACE_DIR"):
        kw["tmpdir"] = os.environ["KERNEL_TRACE_DIR"]
    old = nc.m
    nc.m = get_hw_module(nc.m)
    try:
        res = run_bass_kernel_spmd(nc, in_maps, core_ids=list(range(N_CORES)),
                                   **kw)
    finally:
        nc.m = old
    kernel._last = res

    S, T = consts["S"], consts["T"]
    out = np.empty(N_FULL, F32)
    for c in range(N_CORES):
        oc = res.results[c]["out"]          # [N_ST, 128, B, 7] f16
        oc = oc.transpose(0, 2, 1, 3).reshape(-1)[:N_CORE].astype(F32)
        out[c * N_CORE:(c + 1) * N_CORE] = F32(S) * oc + F32(T)
    return out


# revision 59
# speedup vs baseline: 1.0158x; 1.0158x over previous
"""BinaryTreeRNN Trainium2 kernel — 8-core data-parallel, fp16 pipeline.

Contract: kernel(**inputs) takes FULL unsharded inputs (x [4M,16] f32 plus tiny
tree params) and returns the FULL [4M] f32 output.

Design (per core, N_core = 500k samples, padded to 501760 = 560 blocks x 896):
  * Host folds tree params (float64):  softmax(om) -> per-node (A,P,R,phi,B);
    the combine  o = A*s + R*sin(s+phi) + P*l*r + B  is refactored as
      o = HL*HR + R*sin(theta) + const,   HL = c_hl*(P*l+A), HR = c_hr*(r+A/P)
    (factored quadratic absorbs the linear A*s term).  Stored values carry
    affine maps  true = S*stored + T  folded into the next level's constants;
    per-level power-of-2 scales keep everything in fp16 range.
  * PE: per block one fp16 matmul  out[p,c] = sum_k xt[k,p]*wp[k,c] producing
    12 funcs x 7 slots = 84 cols: HL3/HR3 (4 nodes, L2-pair-permuted) and
    sc3 = (s3+phi3)/2pi.  Bias via two constant rows (112=hi,113=lo).
  * Tree on DVE/ACT in fp16.  Sin range reduction via write-rounding magic:
    ACT copies sc3+1536 PSUM->fp16 (the fp16 write rounds to integer+1536),
    then one scalar_tensor_tensor recovers f = round(sc)-sc from the
    full-precision PSUM sc3; L2 rounds in fp16 (ts_add +1536 / -1536), L1 in
    fp32 (phases exceed fp16's +/-512 magic range).  Per-node constants ride
    broadcast-AP (stride-0) tensor_tensor operands; hr_sc is forced to 1 by
    scale choice so HR needs only a bias add.
  * Emission is software-pipelined: group g's wide-phase chunks interleave
    with group g+1's DMA/matmul/PSUM-evacuation so the in-order engines
    always have ready work; first/last groups are small (ramp/tail).
"""

import os
import sys

for _p in ("/opt/trn_rl_repo", "/root/.axon_site/_ro/trn_rl_repo"):
    if os.path.isdir(_p) and _p not in sys.path:
        sys.path.append(_p)

import numpy as np

N_FULL = 4_000_000
V = 16
N_CORES = 8
N_CORE = N_FULL // N_CORES          # 500_000
SLOTS = 7
BLK = 128 * SLOTS                   # 896
N_BLOCKS = 560
N_PAD = N_BLOCKS * BLK              # 501_760
B = 16                              # blocks per supertile
N_ST = N_BLOCKS // B                # 35
GROUP = 11                          # supertiles per group

TWO_PI = 2.0 * np.pi
M16 = 1536.0                        # fp16 round-to-int magic
PERM = [0, 2, 1, 3]                 # L3 node order: L2 pairs contiguous

F16 = np.float16
F32 = np.float32
F64 = np.float64


def _softmax64(om):
    e = np.exp(om.astype(F64) - om.astype(F64).max(-1, keepdims=True))
    return e / e.sum(-1, keepdims=True)


def _lvl(w, b, om):
    sm = _softmax64(om)
    w64 = w.astype(F64)
    A = w64 * sm[:, 0]
    S = w64 * sm[:, 1]
    C = w64 * sm[:, 2]
    P = w64 * sm[:, 3]
    return dict(A=A, B=b.astype(F64), P=P, R=np.hypot(S, C),
                phi=np.arctan2(C, S))


def _pow2(v):
    return float(2.0 ** np.round(np.log2(max(abs(float(v)), 1e-30))))


def _fold(leaf_w, leaf_b, w1, b1, om1, w2, b2, om2, w3, b3, om3, x_sample):
    """float64 constant folding -> (wp fp16 [128,84], consts dict)."""
    L3 = _lvl(w3, b3, om3)
    L2 = _lvl(w2, b2, om2)
    L1 = _lvl(w1, b1, om1)
    lw = leaf_w.astype(F64)
    lb = leaf_b.astype(F64)
    h = (x_sample.astype(F64) @ lw.T + lb).T      # [8, M]

    def calib(vals, target=2.0):
        return _pow2(target / (np.sqrt((vals ** 2).mean()) + 1e-30))

    # ---- L3 ----
    n3 = []
    o3t = []
    for n in range(4):
        A, P, R, phi, Bc = (L3[k][n] for k in ("A", "P", "R", "phi", "B"))
        l, r = h[2 * n], h[2 * n + 1]
        c_hl = calib(P * l + A)
        c_hr = calib(r + A / P)
        o3t.append(A * (l + r) + R * np.sin(l + r + phi) + P * l * r + Bc)
        n3.append(dict(A=A, P=P, R=R, phi=phi, B=Bc, c_hl=c_hl, c_hr=c_hr,
                       wl=lw[2 * n], wr=lw[2 * n + 1], bl=lb[2 * n],
                       br=lb[2 * n + 1]))
    # shared scale fixed to exactly 1/2pi: stored o3 sums ARE the L2
    # phase in periods, so the L2 sin path needs no rescale op
    cc = 1.0 / TWO_PI
    for d in n3:
        d["c_hr"] *= cc / (d["c_hl"] * d["c_hr"])
        d["c"] = cc
        d["S"] = 1.0 / cc
        d["T"] = d["B"] - d["A"] ** 2 / d["P"]

    # ---- L2 ----  (c_hr forced so hr_sc == 1: HR2 = o3r_stored + hr_b,
    # no multiply needed; c_hl carries all the pow2 balance freedom)
    cc3 = cc
    c_hl_raw = []
    o2t = []
    for m in range(2):
        A, P, R, phi, Bc = (L2[k][m] for k in ("A", "P", "R", "phi", "B"))
        l, r = o3t[2 * m], o3t[2 * m + 1]
        c_hl_raw.append(calib(P * l + A))
        o2t.append(A * (l + r) + R * np.sin(l + r + phi) + P * l * r + Bc)
    cc2 = 1.0 / TWO_PI                  # L2 stored scale also exactly 1/2pi
    n2 = []
    for m in range(2):
        A, P, R, phi, Bc = (L2[k][m] for k in ("A", "P", "R", "phi", "B"))
        cl, cr = n3[2 * m], n3[2 * m + 1]
        c_hr = cc3                      # -> hr_sc = cr["S"]*c_hr = 1
        c_hl = cc2 / cc3
        th_b = cl["T"] + cr["T"] + phi
        n2.append(dict(
            A=A, P=P, R=R, phi=phi, B=Bc, c_hl=c_hl, c_hr=c_hr,
            hl_sc=P * cl["S"] * c_hl, hl_b=(A + P * cl["T"]) * c_hl,
            hr_sc=1.0, hr_b=(cr["T"] + A / P) * c_hr,
            sc_sc=cl["S"] / TWO_PI,
            dfrac=(th_b / TWO_PI) - np.round(th_b / TWO_PI),
            c=cc2, S=1.0 / cc2, T=Bc - A ** 2 / P,
        ))

    # ---- L1 ----  (same hr_sc == 1 construction)
    A, P, R, phi, Bc = (L1[k][0] for k in ("A", "P", "R", "phi", "B"))
    cl, cr = n2
    l, r = o2t
    c_hr = cc2
    c_hl = calib(P * l + A)
    th_b = cl["T"] + cr["T"] + phi
    n1 = dict(
        A=A, P=P, R=R, phi=phi, B=Bc, c_hl=c_hl, c_hr=c_hr,
        hl_sc=P * cl["S"] * c_hl, hl_b=(A + P * cl["T"]) * c_hl,
        hr_sc=1.0, hr_b=(cr["T"] + A / P) * c_hr,
        sc_sc=cl["S"] / TWO_PI,
        dfrac=(th_b / TWO_PI) - np.round(th_b / TWO_PI),
        c=c_hl * c_hr,
    )
    n1["S"] = 1.0 / n1["c"]
    n1["T"] = Bc - A ** 2 / P
    # sanity: HR tensors are o_stored + hr_b; biases must stay in fp16 range
    assert abs(n1["hr_b"]) < 3e4 and all(abs(d["hr_b"]) < 3e4 for d in n2)
    # stored o2 = o2_true/2pi must stay inside fp16 range incl. tails
    assert max(np.abs(v).max() for v in o2t) * 2.0 / TWO_PI < 6e4, \
        "stored o2 exceeds fp16 range with S2 = 2pi"

    # ---- PE weight matrix [128, 84]: col 7j+a ----
    wp = np.zeros((128, 84), F64)
    for j, n in enumerate(PERM):
        d = n3[n]
        cols = [
            (j, d["wl"] * d["P"] * d["c_hl"],
             (d["P"] * d["bl"] + d["A"]) * d["c_hl"]),
            (4 + j, d["wr"] * d["c_hr"], (d["br"] + d["A"] / d["P"]) * d["c_hr"]),
            (8 + j, (d["wl"] + d["wr"]) / TWO_PI,
             (d["bl"] + d["br"] + d["phi"]) / TWO_PI),
        ]
        for jj, wv, bias in cols:
            for a_ in range(SLOTS):
                wp[16 * a_:16 * a_ + 16, 7 * jj + a_] = wv
                bh = np.float16(bias)
                wp[112, 7 * jj + a_] = bh
                wp[113, 7 * jj + a_] = np.float16(bias - float(bh))
    wp16 = wp.astype(F16)

    # packed broadcast-constant columns [119]:
    #  0:28  r3bc   28:42 r2bc   42:56 d2bc(dfrac)  56:70 hs2  70:84 hb2
    #  84:98 rs2   98:112 rb2   112:119 r1bc
    cst = np.zeros(119, F64)
    for j, n in enumerate(PERM):
        cst[7 * j:7 * j + 7] = n3[n]["R"] * n3[n]["c"]
    for m in range(2):
        s = slice(28 + 7 * m, 35 + 7 * m)
        cst[s.start:s.stop] = n2[m]["R"] * n2[m]["c"]
        cst[s.start + 14:s.stop + 14] = n2[m]["dfrac"]
        cst[s.start + 28:s.stop + 28] = n2[m]["hl_sc"]
        cst[s.start + 42:s.stop + 42] = n2[m]["hl_b"]
        cst[s.start + 56:s.stop + 56] = n2[m]["hr_sc"]
        cst[s.start + 70:s.stop + 70] = n2[m]["hr_b"]
    cst[112:119] = n1["R"] * n1["c"]
    cst16 = np.broadcast_to(cst.astype(F16), (128, 119)).copy()

    # fp16 magic rounding at L3/L2 requires |theta|/2pi well below 512
    mx3 = max(np.abs(h[2 * n] + h[2 * n + 1] + n3[n]["phi"]).max()
              for n in range(4)) / TWO_PI
    mx2 = max(np.abs(o3t[2 * m] + o3t[2 * m + 1] + n2[m]["phi"]).max()
              for m in range(2)) / TWO_PI
    # 2.5x extrapolation subsample-max -> full-N max; beyond 512 a tail
    # sample gets a bounded sign-flipped sin (negligible in L2 norm),
    # beyond ~2048 sin output explodes -> hard error.
    assert mx3 * 2.5 < 500 and mx2 * 2.5 < 2000, \
        f"fp16 sin-magic range exceeded: sc3 max {mx3:.1f}, sc2 max {mx2:.1f}"

    consts = dict(L2=n2, L1=n1, S=n1["S"], T=n1["T"])
    return wp16, cst16, consts


def _pack_x(x_shard, n_st=N_ST, b_blocks=B):
    """[n,16] f32 -> fp16 [n_st, 112, b_blocks*128] stationary rows."""
    npad = n_st * b_blocks * BLK
    xs = np.empty((npad, V), F32)
    xs[:len(x_shard)] = x_shard
    xs[len(x_shard):] = 1.0
    a = xs.reshape(n_st, b_blocks, 128, SLOTS, V)      # [st, b, p, a, v]
    xt = a.transpose(0, 3, 4, 1, 2).reshape(n_st, 112, b_blocks * 128)
    return np.ascontiguousarray(xt, dtype=F16)


_PROGRAM_CACHE = {}


def _build_program(n_st=N_ST, b_blocks=B, group=GROUP):
    import json
    key = (n_st, b_blocks, group,
           json.dumps(_build_program.consts, sort_keys=True, default=str))
    if key in _PROGRAM_CACHE:
        return _PROGRAM_CACHE[key]

    import concourse.bass as bass
    import concourse.tile as tile
    from concourse import bacc, mybir
    from contextlib import ExitStack

    f32 = mybir.dt.float32
    f16 = mybir.dt.float16
    Sin = mybir.ActivationFunctionType.Sin
    Ident = mybir.ActivationFunctionType.Identity
    sub = mybir.AluOpType.subtract
    mult = mybir.AluOpType.mult
    addop = mybir.AluOpType.add

    C = _build_program.consts
    n2, n1 = C["L2"], C["L1"]
    M32 = float(1.5 * 2.0 ** 23)

    nc = bacc.Bacc("TRN2", target_bir_lowering=False, debug=False,
                   num_devices=N_CORES)
    xh_d = nc.dram_tensor("xh", [n_st, 112, b_blocks * 128], f16,
                          kind="ExternalInput")
    wp_d = nc.dram_tensor("wp", [128, 84], f16, kind="ExternalInput")
    cst_d = nc.dram_tensor("cst", [128, 119], f16, kind="ExternalInput")
    out_d = nc.dram_tensor("out", [n_st, 128, b_blocks, SLOTS], f16,
                           kind="ExternalOutput")

    def reg_const(v):
        v = float(F32(v))
        if (f32, v) not in nc.const_aps.aps:
            t = nc.alloc_sbuf_tensor(
                f"constx-{len(nc.const_aps.aps)}", [128, 1], f32)
            nc.gpsimd.memset(t.ap(), v)
            nc.const_aps.aps[(f32, v)] = t.ap()

    reg_const(0.0)
    reg_const(M16)
    for d in n2:
        reg_const(TWO_PI * d["dfrac"])
    warm = nc.alloc_sbuf_tensor("sinwarm", [128, 1], f32)
    nc.gpsimd.memset(warm.ap(), 0.0)
    nc.all_engine_barrier()
    # warm up the Sin spline table set as the first ACT op: the ~2.7us
    # ACT_TABLE_LOAD overlaps the initial DMAs/matmuls instead of sitting
    # on the first group's critical path
    nc.scalar.activation(warm.ap(), warm.ap(), Sin, bias=0.0, scale=1.0)

    with tile.TileContext(nc) as tc:
        with ExitStack() as ctx:
            const_pool = ctx.enter_context(tc.tile_pool(name="const", bufs=1))
            xpool = ctx.enter_context(tc.tile_pool(name="x", bufs=1))
            ppool = ctx.enter_context(
                tc.tile_pool(name="ps", bufs=2, space=bass.MemorySpace.PSUM))
            g2pool = ctx.enter_context(tc.tile_pool(name="g2", bufs=2))
            g1pool = ctx.enter_context(tc.tile_pool(name="g1", bufs=1))

            wp = const_pool.tile([128, 84], f16)
            nc.sync.dma_start(wp[:], wp_d[:])
            cst = const_pool.tile([128, 119], f16)
            nc.sync.dma_start(cst[:], cst_d[:])

            def bc(lo, hi, q):
                return cst[:, lo:hi].unsqueeze(1).broadcast_to(
                    (128, q, hi - lo))

            xts = []
            for i in range(3):
                t = xpool.tile([128, b_blocks * 128], f16, name=f"xt{i}",
                               tag=f"xt{i}")
                # start partition must be a multiple of 32; rows 96:112 are
                # rewritten by every x DMA, rows 112:128 stay constant 1.0
                nc.gpsimd.memset(t[96:128, :], 1.0)
                xts.append(t)

            def alloc_group():
                """Seg-phase-filled tiles (double-buffered across groups)."""
                tt = {}
                for nm in ("g3", "k3", "hl3", "f3"):
                    t = g2pool.tile([128, group * b_blocks * 28], f16,
                                    name=nm, tag=nm)
                    tt[nm] = (t, t[:].rearrange("p (q c) -> p q c", c=28))
                return tt

            def emit_seg(tiles, st):
                xt = xts[st % 3]
                nc.sync.dma_start(xt[0:112, :], xh_d[st])
                ps = ppool.tile([128, b_blocks * 128], f32)
                for b in range(b_blocks):
                    nc.tensor.matmul(ps[:, 128 * b:128 * b + 84],
                                     xt[:, 128 * b:128 * b + 128],
                                     wp[:], start=True, stop=True)
                psv = ps[:].rearrange("p (b c) -> p b c", c=128)
                seg = emit_seg.idx
                emit_seg.idx += 1
                ssl = slice(seg * b_blocks, (seg + 1) * b_blocks)
                hl3v = tiles["hl3"][1]
                k3v = tiles["k3"][1]
                # PSUM evacuation (one PSUM operand per op); k3/f3 first so
                # DVE unblocks after a single ACT copy:
                #   k3  <- sc3 + M16 (fp16 write rounds to integer+M16)
                #   f3  <- (k3 - M16) - sc3[psum]
                #   hl3 <- HL cols;  g3 <- hl3 * HR[psum]
                nc.scalar.activation(k3v[:, ssl, :], psv[:, :, 56:84],
                                     Ident, bias=M16, scale=1.0)
                nc.vector.scalar_tensor_tensor(
                    tiles["f3"][1][:, ssl, :], k3v[:, ssl, :], M16,
                    psv[:, :, 56:84], sub, sub)
                nc.scalar.activation(hl3v[:, ssl, :], psv[:, :, 0:28],
                                     Ident, bias=0.0, scale=1.0)
                nc.vector.tensor_tensor(tiles["g3"][1][:, ssl, :],
                                        hl3v[:, ssl, :],
                                        psv[:, :, 28:56], mult)

            def wide_chunks(tiles, glen, st0):
                """Yield the wide-phase as a list of closures (chunks)."""
                q = glen * b_blocks
                qsl = slice(0, q)
                qf3, qf2, qf1 = q * 28, q * 14, q * 7
                g3 = tiles["g3"][0]
                f3 = tiles["f3"][0]

                def gt(cols, nm, dt=f16):
                    t = g1pool.tile([128, group * b_blocks * cols], dt,
                                    name=nm, tag=nm)
                    return t, t[:].rearrange("p (q c) -> p q c", c=cols)

                st_ = {}

                def c_t3u3():
                    t3, t3v = gt(28, "t3")
                    nc.scalar.activation(t3[:, 0:qf3], f3[:, 0:qf3], Sin,
                                         bias=0.0, scale=float(F32(-TWO_PI)))
                    u3, u3v = gt(28, "u3")
                    nc.vector.tensor_tensor(u3v[:, qsl, :], t3v[:, qsl, :],
                                            bc(0, 28, q), mult)
                    st_["u3"] = u3

                def c_o3():
                    o3, o3v = gt(28, "o3")
                    nc.vector.tensor_tensor(o3[:, 0:qf3], g3[:, 0:qf3],
                                            st_["u3"][:, 0:qf3], addop)
                    st_["o3"] = (o3, o3v)

                def c_s2():
                    # stored o3 is scaled exactly 1/2pi: s2 IS the phase in
                    # periods; round directly (per-node dfrac in the magic)
                    o3, o3v = st_["o3"]
                    st_["l2v"] = o3v[:, qsl, 0:14]
                    st_["r2v"] = o3v[:, qsl, 14:28]
                    s2, s2v = gt(14, "s2")
                    nc.vector.tensor_tensor(s2v[:, qsl, :], st_["l2v"],
                                            st_["r2v"], addop)
                    st_["s2"] = (s2, s2v)

                def c_k2():
                    s2, s2v = st_["s2"]
                    k2, k2v = gt(14, "k2")
                    for m in range(2):
                        d = n2[m]
                        nc.vector.tensor_scalar_add(
                            k2v[:, qsl, 7 * m:7 * m + 7],
                            s2v[:, qsl, 7 * m:7 * m + 7],
                            float(F32(M16 + d["dfrac"])))
                    nc.vector.tensor_scalar_sub(k2[:, 0:qf2], k2[:, 0:qf2],
                                                M16)
                    st_["k2"] = k2

                def c_f2():
                    s2, s2v = st_["s2"]
                    f2, f2v = gt(14, "f2")
                    nc.vector.tensor_tensor(f2[:, 0:qf2],
                                            st_["k2"][:, 0:qf2],
                                            s2[:, 0:qf2], sub)
                    t2, t2v = gt(14, "t2")
                    for m in range(2):
                        d = n2[m]
                        nc.scalar.activation(
                            t2v[:, qsl, 7 * m:7 * m + 7],
                            f2v[:, qsl, 7 * m:7 * m + 7], Sin,
                            bias=float(F32(TWO_PI * d["dfrac"])),
                            scale=float(F32(-TWO_PI)))
                    u2, u2v = gt(14, "u2")
                    nc.vector.tensor_tensor(u2v[:, qsl, :], t2v[:, qsl, :],
                                            bc(28, 42, q), mult)
                    st_["u2"] = u2

                def c_HL2():
                    HL2, HL2v = gt(14, "HL2")
                    nc.vector.tensor_tensor(HL2v[:, qsl, :], st_["l2v"],
                                            bc(56, 70, q), mult)
                    for m in range(2):
                        d = n2[m]
                        sl7 = (slice(None), qsl, slice(7 * m, 7 * m + 7))
                        nc.vector.tensor_scalar_add(HL2v[sl7], HL2v[sl7],
                                                    float(F32(d["hl_b"])))
                    st_["HL2"] = HL2

                def c_HR2():
                    HR2, HR2v = gt(14, "HR2")
                    for m in range(2):
                        d = n2[m]
                        sl7 = (slice(None), qsl, slice(7 * m, 7 * m + 7))
                        nc.vector.tensor_scalar_add(
                            HR2v[sl7], st_["r2v"][:, :, 7 * m:7 * m + 7],
                            float(F32(d["hr_b"])))
                    g2t, _ = gt(14, "g2")
                    nc.vector.tensor_tensor(g2t[:, 0:qf2],
                                            st_["HL2"][:, 0:qf2],
                                            HR2[:, 0:qf2], mult)
                    st_["g2"] = g2t

                def c_o2():
                    o2, o2v = gt(14, "o2")
                    nc.vector.tensor_tensor(o2[:, 0:qf2], st_["g2"][:, 0:qf2],
                                            st_["u2"][:, 0:qf2], addop)
                    st_["o2"] = (o2, o2v)

                def c_s1():
                    o2, o2v = st_["o2"]
                    st_["l1v"] = o2v[:, qsl, 0:7]
                    st_["r1v"] = o2v[:, qsl, 7:14]
                    d = n1
                    s1, s1v = gt(7, "s1")
                    nc.vector.tensor_tensor(s1v[:, qsl, :], st_["l1v"],
                                            st_["r1v"], addop)
                    # sc_sc == 1 (stored scale is 1/2pi): the f32 convert
                    # and dfrac shift merge into one ts_add
                    sc1, _ = gt(7, "sc1", dt=f32)
                    nc.vector.tensor_scalar_add(sc1[:, 0:qf1], s1[:, 0:qf1],
                                                float(F32(d["dfrac"])))
                    st_["sc1"] = sc1

                def c_f1():
                    d = n1
                    sc1 = st_["sc1"]
                    k1, _ = gt(7, "k1", dt=f32)
                    nc.vector.tensor_scalar_add(k1[:, 0:qf1], sc1[:, 0:qf1],
                                                M32)
                    f1, _ = gt(7, "f1")
                    nc.vector.scalar_tensor_tensor(
                        f1[:, 0:qf1], k1[:, 0:qf1], M32, sc1[:, 0:qf1],
                        sub, sub)
                    t1, t1v = gt(7, "t1")
                    nc.scalar.activation(t1[:, 0:qf1], f1[:, 0:qf1], Sin,
                                         bias=0.0, scale=float(F32(-TWO_PI)))
                    st_["t1"] = (t1, t1v)

                def c_g1():
                    d = n1
                    HL1, HL1v = gt(7, "HL1")
                    nc.vector.tensor_scalar_mul(HL1v[:, qsl, :], st_["l1v"],
                                                float(F32(d["hl_sc"])))
                    nc.vector.tensor_scalar_add(HL1[:, 0:qf1], HL1[:, 0:qf1],
                                                float(F32(d["hl_b"])))
                    HR1, HR1v = gt(7, "HR1")
                    nc.vector.tensor_scalar_add(HR1v[:, qsl, :], st_["r1v"],
                                                float(F32(d["hr_b"])))
                    g1t, _ = gt(7, "g1")
                    nc.vector.tensor_tensor(g1t[:, 0:qf1], HL1[:, 0:qf1],
                                            HR1[:, 0:qf1], mult)
                    st_["g1"] = g1t

                def c_yo():
                    # L1 has a single node: R is one scalar -> fused stt
                    t1, t1v = st_["t1"]
                    yo, _ = gt(7, "yo")
                    nc.vector.scalar_tensor_tensor(
                        yo[:, 0:qf1], t1[:, 0:qf1],
                        float(F32(n1["R"] * n1["c"])), st_["g1"][:, 0:qf1],
                        mult, addop)
                    dst = out_d[st0:st0 + glen].transpose([1, 0, 2, 3])
                    yo4 = yo[:, 0:qf1].rearrange("p (g b a) -> p g b a",
                                                 g=glen, a=SLOTS)
                    nc.sync.dma_start(dst, yo4)

                return [c_t3u3, c_o3, c_s2, c_k2, c_f2, c_HL2, c_HR2,
                        c_o2, c_s1, c_f1, c_g1, c_yo]

            # group sizes: small first group shortens the pipeline fill
            # ramp; small last group shortens the un-overlapped tail
            glens = []
            rem = n_st
            if n_st > group + 5:
                glens.append(2)
                rem -= 2
                tail = 2 if rem % group != 1 else 3
                if rem % group not in (0, 1):
                    tail = rem % group if rem % group <= 2 else 2
                while rem > tail:
                    glens.append(min(group, rem - tail))
                    rem -= glens[-1]
                glens.append(rem)
                rem = 0
            while rem > 0:
                glens.append(min(group, rem))
                rem -= glens[-1]

            # software pipeline: interleave group g's wide chunks with
            # group g+1's seg ops so the (in-order) engines always have
            # ready work queued.
            prev = None
            st0 = 0
            for glen in glens:
                tiles = alloc_group()
                emit_seg.idx = 0
                segs = list(range(st0, st0 + glen))
                if prev is None:
                    for st in segs:
                        emit_seg(tiles, st)
                else:
                    # chunk-first: the first chunk (t3/u3) depends only on
                    # the previous group and keeps ACT/DVE busy while the
                    # new group's DMAs/matmuls start.
                    chunks = wide_chunks(*prev)
                    si = ci = 0
                    while ci < len(chunks) or si < len(segs):
                        if ci < len(chunks):
                            chunks[ci]()
                            ci += 1
                        if si < len(segs) and (ci * len(segs)
                                               >= si * len(chunks)):
                            emit_seg(tiles, segs[si])
                            si += 1
                prev = (tiles, glen, st0)
                st0 += glen
            for c in wide_chunks(*prev):
                c()

    nc.compile()
    _PROGRAM_CACHE[key] = nc
    return nc


def kernel(x, leaf_w, leaf_b, w1, b1, om1, w2, b2, om2, w3, b3, om3):
    from concourse.bass_interp import get_hw_module
    from concourse.bass_utils import run_bass_kernel_spmd

    x = np.ascontiguousarray(x, dtype=F32)
    wp, cst, consts = _fold(
        leaf_w, leaf_b, w1, b1, om1, w2, b2, om2, w3, b3, om3,
        x[:: max(1, N_FULL // 4096)][:4096])
    _build_program.consts = consts
    nc = _build_program()

    in_maps = []
    for c in range(N_CORES):
        xh = _pack_x(x[c * N_CORE:(c + 1) * N_CORE])
        in_maps.append({"xh": xh, "wp": wp, "cst": cst})

    kw = {}
    if os.environ.get("KERNEL_TRACE_DIR"):
        kw["tmpdir"] = os.environ["KERNEL_TRACE_DIR"]
    old = nc.m
    nc.m = get_hw_module(nc.m)
    try:
        res = run_bass_kernel_spmd(nc, in_maps, core_ids=list(range(N_CORES)),
                                   **kw)
    finally:
        nc.m = old
    kernel._last = res

    S, T = consts["S"], consts["T"]
    out = np.empty(N_FULL, F32)
    for c in range(N_CORES):
        oc = res.results[c]["out"]          # [N_ST, 128, B, 7] f16
        oc = oc.transpose(0, 2, 1, 3).reshape(-1)[:N_CORE].astype(F32)
        out[c * N_CORE:(c + 1) * N_CORE] = F32(S) * oc + F32(T)
    return out


# revision 63
# speedup vs baseline: 1.1307x; 1.1131x over previous
"""BinaryTreeRNN Trainium2 kernel — 8-core data-parallel, fp16 pipeline.

Contract: kernel(**inputs) takes FULL unsharded inputs (x [4M,16] f32 plus tiny
tree params) and returns the FULL [4M] f32 output.

Design (per core, N_core = 500k samples, padded to 501760 = 560 blocks x 896):
  * Host folds tree params (float64):  softmax(om) -> per-node (A,P,R,phi,B);
    the combine  o = A*s + R*sin(s+phi) + P*l*r + B  is refactored as
      o = HL*HR + R*sin(theta) + const,   HL = c_hl*(P*l+A), HR = c_hr*(r+A/P)
    (factored quadratic absorbs the linear A*s term).  Stored values carry
    affine maps  true = S*stored + T  folded into the next level's constants;
    per-level power-of-2 scales keep everything in fp16 range.
  * PE: per block one fp16 matmul  out[p,c] = sum_k xt[k,p]*wp[k,c] producing
    12 funcs x 7 slots = 84 cols: HL3/HR3 (4 nodes, L2-pair-permuted) and
    sc3 = (s3+phi3)/2pi.  Bias via two constant rows (112=hi,113=lo).
  * Tree on DVE/ACT in fp16.  Sin range reduction via write-rounding magic:
    ACT copies sc3+1536 PSUM->fp16 (the fp16 write rounds to integer+1536),
    then one scalar_tensor_tensor recovers f = round(sc)-sc from the
    full-precision PSUM sc3; L2 rounds in fp16 (ts_add +1536 / -1536), L1 in
    fp32 (phases exceed fp16's +/-512 magic range).  Per-node constants ride
    broadcast-AP (stride-0) tensor_tensor operands; hr_sc is forced to 1 by
    scale choice so HR needs only a bias add.
  * Emission is software-pipelined: group g's wide-phase chunks interleave
    with group g+1's DMA/matmul/PSUM-evacuation so the in-order engines
    always have ready work; first/last groups are small (ramp/tail).
"""

import os
import sys

for _p in ("/opt/trn_rl_repo", "/root/.axon_site/_ro/trn_rl_repo"):
    if os.path.isdir(_p) and _p not in sys.path:
        sys.path.append(_p)

import numpy as np

N_FULL = 4_000_000
V = 16
N_CORES = 8
N_CORE = N_FULL // N_CORES          # 500_000
SLOTS = 7
BLK = 128 * SLOTS                   # 896
N_BLOCKS = 560
N_PAD = N_BLOCKS * BLK              # 501_760
B = 16                              # blocks per supertile
N_ST = N_BLOCKS // B                # 35
GROUP = 11                          # supertiles per group

TWO_PI = 2.0 * np.pi
M16 = 1536.0                        # fp16 round-to-int magic
PERM = [0, 2, 1, 3]                 # L3 node order: L2 pairs contiguous

F16 = np.float16
F32 = np.float32
F64 = np.float64


def _softmax64(om):
    e = np.exp(om.astype(F64) - om.astype(F64).max(-1, keepdims=True))
    return e / e.sum(-1, keepdims=True)


def _lvl(w, b, om):
    sm = _softmax64(om)
    w64 = w.astype(F64)
    A = w64 * sm[:, 0]
    S = w64 * sm[:, 1]
    C = w64 * sm[:, 2]
    P = w64 * sm[:, 3]
    return dict(A=A, B=b.astype(F64), P=P, R=np.hypot(S, C),
                phi=np.arctan2(C, S))


def _pow2(v):
    return float(2.0 ** np.round(np.log2(max(abs(float(v)), 1e-30))))


def _fold(leaf_w, leaf_b, w1, b1, om1, w2, b2, om2, w3, b3, om3, x_sample):
    """float64 constant folding -> (wp fp16 [128,84], consts dict)."""
    L3 = _lvl(w3, b3, om3)
    L2 = _lvl(w2, b2, om2)
    L1 = _lvl(w1, b1, om1)
    lw = leaf_w.astype(F64)
    lb = leaf_b.astype(F64)
    h = (x_sample.astype(F64) @ lw.T + lb).T      # [8, M]

    def calib(vals, target=2.0):
        return _pow2(target / (np.sqrt((vals ** 2).mean()) + 1e-30))

    # ---- L3 ----
    n3 = []
    o3t = []
    for n in range(4):
        A, P, R, phi, Bc = (L3[k][n] for k in ("A", "P", "R", "phi", "B"))
        l, r = h[2 * n], h[2 * n + 1]
        c_hl = calib(P * l + A)
        c_hr = calib(r + A / P)
        o3t.append(A * (l + r) + R * np.sin(l + r + phi) + P * l * r + Bc)
        n3.append(dict(A=A, P=P, R=R, phi=phi, B=Bc, c_hl=c_hl, c_hr=c_hr,
                       wl=lw[2 * n], wr=lw[2 * n + 1], bl=lb[2 * n],
                       br=lb[2 * n + 1]))
    # shared scale fixed to exactly 1/2pi: stored o3 sums ARE the L2
    # phase in periods, so the L2 sin path needs no rescale op
    cc = 1.0 / TWO_PI
    for d in n3:
        d["c_hr"] *= cc / (d["c_hl"] * d["c_hr"])
        d["c"] = cc
        d["S"] = 1.0 / cc
        d["T"] = d["B"] - d["A"] ** 2 / d["P"]

    # ---- L2 ----  (c_hr forced so hr_sc == 1: HR2 = o3r_stored + hr_b,
    # no multiply needed; c_hl carries all the pow2 balance freedom)
    cc3 = cc
    c_hl_raw = []
    o2t = []
    for m in range(2):
        A, P, R, phi, Bc = (L2[k][m] for k in ("A", "P", "R", "phi", "B"))
        l, r = o3t[2 * m], o3t[2 * m + 1]
        c_hl_raw.append(calib(P * l + A))
        o2t.append(A * (l + r) + R * np.sin(l + r + phi) + P * l * r + Bc)
    cc2 = 1.0 / TWO_PI                  # L2 stored scale also exactly 1/2pi
    n2 = []
    for m in range(2):
        A, P, R, phi, Bc = (L2[k][m] for k in ("A", "P", "R", "phi", "B"))
        cl, cr = n3[2 * m], n3[2 * m + 1]
        c_hr = cc3                      # -> hr_sc = cr["S"]*c_hr = 1
        c_hl = cc2 / cc3
        th_b = cl["T"] + cr["T"] + phi
        n2.append(dict(
            A=A, P=P, R=R, phi=phi, B=Bc, c_hl=c_hl, c_hr=c_hr,
            hl_sc=P * cl["S"] * c_hl, hl_b=(A + P * cl["T"]) * c_hl,
            hr_sc=1.0, hr_b=(cr["T"] + A / P) * c_hr,
            sc_sc=cl["S"] / TWO_PI,
            dfrac=(th_b / TWO_PI) - np.round(th_b / TWO_PI),
            c=cc2, S=1.0 / cc2, T=Bc - A ** 2 / P,
        ))

    # ---- L1 ----  (same hr_sc == 1 construction)
    A, P, R, phi, Bc = (L1[k][0] for k in ("A", "P", "R", "phi", "B"))
    cl, cr = n2
    l, r = o2t
    c_hr = cc2
    c_hl = calib(P * l + A)
    th_b = cl["T"] + cr["T"] + phi
    n1 = dict(
        A=A, P=P, R=R, phi=phi, B=Bc, c_hl=c_hl, c_hr=c_hr,
        hl_sc=P * cl["S"] * c_hl, hl_b=(A + P * cl["T"]) * c_hl,
        hr_sc=1.0, hr_b=(cr["T"] + A / P) * c_hr,
        sc_sc=cl["S"] / TWO_PI,
        dfrac=(th_b / TWO_PI) - np.round(th_b / TWO_PI),
        c=c_hl * c_hr,
    )
    n1["S"] = 1.0 / n1["c"]
    n1["T"] = Bc - A ** 2 / P
    # sanity: HR tensors are o_stored + hr_b; biases must stay in fp16 range
    assert abs(n1["hr_b"]) < 3e4 and all(abs(d["hr_b"]) < 3e4 for d in n2)
    # stored o2 = o2_true/2pi must stay inside fp16 range incl. tails
    assert max(np.abs(v).max() for v in o2t) * 2.0 / TWO_PI < 6e4, \
        "stored o2 exceeds fp16 range with S2 = 2pi"

    # ---- PE weight matrix [128, 84]: col 7j+a ----
    wp = np.zeros((128, 84), F64)
    for j, n in enumerate(PERM):
        d = n3[n]
        cols = [
            (j, d["wl"] * d["P"] * d["c_hl"],
             (d["P"] * d["bl"] + d["A"]) * d["c_hl"]),
            (4 + j, d["wr"] * d["c_hr"], (d["br"] + d["A"] / d["P"]) * d["c_hr"]),
            (8 + j, (d["wl"] + d["wr"]) / TWO_PI,
             (d["bl"] + d["br"] + d["phi"]) / TWO_PI),
        ]
        for jj, wv, bias in cols:
            for a_ in range(SLOTS):
                wp[16 * a_:16 * a_ + 16, 7 * jj + a_] = wv
                bh = np.float16(bias)
                wp[112, 7 * jj + a_] = bh
                wp[113, 7 * jj + a_] = np.float16(bias - float(bh))
    wp16 = wp.astype(F16)

    # packed broadcast-constant columns [119]:
    #  0:28  r3bc   28:42 r2bc   42:56 d2bc(dfrac)  56:70 hs2  70:84 hb2
    #  84:98 rs2   98:112 rb2   112:119 r1bc
    cst = np.zeros(119, F64)
    for j, n in enumerate(PERM):
        cst[7 * j:7 * j + 7] = n3[n]["R"] * n3[n]["c"]
    for m in range(2):
        s = slice(28 + 7 * m, 35 + 7 * m)
        cst[s.start:s.stop] = n2[m]["R"] * n2[m]["c"]
        cst[s.start + 14:s.stop + 14] = n2[m]["dfrac"]
        cst[s.start + 28:s.stop + 28] = n2[m]["hl_sc"]
        cst[s.start + 42:s.stop + 42] = n2[m]["hl_b"]
        cst[s.start + 56:s.stop + 56] = n2[m]["hr_sc"]
        cst[s.start + 70:s.stop + 70] = n2[m]["hr_b"]
    cst[112:119] = n1["R"] * n1["c"]
    cst16 = np.broadcast_to(cst.astype(F16), (128, 119)).copy()

    # fp16 magic rounding at L3/L2 requires |theta|/2pi well below 512
    mx3 = max(np.abs(h[2 * n] + h[2 * n + 1] + n3[n]["phi"]).max()
              for n in range(4)) / TWO_PI
    mx2 = max(np.abs(o3t[2 * m] + o3t[2 * m + 1] + n2[m]["phi"]).max()
              for m in range(2)) / TWO_PI
    # 2.5x extrapolation subsample-max -> full-N max; beyond 512 a tail
    # sample gets a bounded sign-flipped sin (negligible in L2 norm),
    # beyond ~2048 sin output explodes -> hard error.
    assert mx3 * 2.5 < 500 and mx2 * 2.5 < 2000, \
        f"fp16 sin-magic range exceeded: sc3 max {mx3:.1f}, sc2 max {mx2:.1f}"

    consts = dict(L2=n2, L1=n1, S=n1["S"], T=n1["T"])
    return wp16, cst16, consts


def _pack_x(x_shard, n_st=N_ST, b_blocks=B):
    """[n,16] f32 -> fp16 [n_st, 112, b_blocks*128] stationary rows."""
    npad = n_st * b_blocks * BLK
    xs = np.empty((npad, V), F32)
    xs[:len(x_shard)] = x_shard
    xs[len(x_shard):] = 1.0
    a = xs.reshape(n_st, b_blocks, 128, SLOTS, V)      # [st, b, p, a, v]
    xt = a.transpose(0, 3, 4, 1, 2).reshape(n_st, 112, b_blocks * 128)
    return np.ascontiguousarray(xt, dtype=F16)


_PROGRAM_CACHE = {}


def _build_program(n_st=N_ST, b_blocks=B, group=GROUP):
    import json
    key = (n_st, b_blocks, group,
           json.dumps(_build_program.consts, sort_keys=True, default=str))
    if key in _PROGRAM_CACHE:
        return _PROGRAM_CACHE[key]

    import concourse.bass as bass
    import concourse.tile as tile
    from concourse import bacc, mybir
    from contextlib import ExitStack

    f32 = mybir.dt.float32
    f16 = mybir.dt.float16
    Sin = mybir.ActivationFunctionType.Sin
    Ident = mybir.ActivationFunctionType.Identity
    sub = mybir.AluOpType.subtract
    mult = mybir.AluOpType.mult
    addop = mybir.AluOpType.add

    C = _build_program.consts
    n2, n1 = C["L2"], C["L1"]
    M32 = float(1.5 * 2.0 ** 23)

    nc = bacc.Bacc("TRN2", target_bir_lowering=False, debug=False,
                   num_devices=N_CORES)
    xh_d = nc.dram_tensor("xh", [n_st, 112, b_blocks * 128], f16,
                          kind="ExternalInput")
    wp_d = nc.dram_tensor("wp", [128, 84], f16, kind="ExternalInput")
    cst_d = nc.dram_tensor("cst", [128, 119], f16, kind="ExternalInput")
    ones_d = nc.dram_tensor("ones", [16, b_blocks * 128], f16,
                            kind="ExternalInput")
    out_d = nc.dram_tensor("out", [n_st, 128, b_blocks, SLOTS], f16,
                           kind="ExternalOutput")

    def reg_const(v):
        v = float(F32(v))
        if (f32, v) not in nc.const_aps.aps:
            t = nc.alloc_sbuf_tensor(
                f"constx-{len(nc.const_aps.aps)}", [128, 1], f32)
            nc.gpsimd.memset(t.ap(), v)
            nc.const_aps.aps[(f32, v)] = t.ap()

    reg_const(0.0)
    reg_const(M16)
    for d in n2:
        reg_const(TWO_PI * d["dfrac"])
    warm = nc.alloc_sbuf_tensor("sinwarm", [128, 1], f32)
    nc.gpsimd.memset(warm.ap(), 0.0)
    nc.all_engine_barrier()
    # warm up the Sin spline table set as the first ACT op: the ~2.7us
    # ACT_TABLE_LOAD overlaps the initial DMAs/matmuls instead of sitting
    # on the first group's critical path
    nc.scalar.activation(warm.ap(), warm.ap(), Sin, bias=0.0, scale=1.0)

    with tile.TileContext(nc) as tc:
        with ExitStack() as ctx:
            const_pool = ctx.enter_context(tc.tile_pool(name="const", bufs=1))
            xpool = ctx.enter_context(tc.tile_pool(name="x", bufs=1))
            ppool = ctx.enter_context(
                tc.tile_pool(name="ps", bufs=2, space=bass.MemorySpace.PSUM))
            g2pool = ctx.enter_context(tc.tile_pool(name="g2", bufs=2))
            g1pool = ctx.enter_context(tc.tile_pool(name="g1", bufs=1))

            wp = const_pool.tile([128, 84], f16)
            nc.sync.dma_start(wp[:], wp_d[:])
            cst = const_pool.tile([128, 119], f16)
            nc.sync.dma_start(cst[:], cst_d[:])

            def bc(lo, hi, q):
                return cst[:, lo:hi].unsqueeze(1).broadcast_to(
                    (128, q, hi - lo))

            xts = []
            for i in range(2):
                t = xpool.tile([128, b_blocks * 128], f16, name=f"xt{i}",
                               tag=f"xt{i}")
                # constant 1.0 rows (112:128) come from DRAM: a DMA write to
                # exactly those partitions (engine memset would need a
                # 32-aligned start and an overlap with the x rows, putting a
                # 1.8us GPSIMD op on the first x-DMA's WAR path)
                nc.sync.dma_start(t[112:128, :], ones_d[:])
                xts.append(t)

            def alloc_group():
                """Seg-phase-filled tiles (double-buffered across groups)."""
                tt = {}
                for nm in ("g3", "k3", "hl3", "f3"):
                    t = g2pool.tile([128, group * b_blocks * 28], f16,
                                    name=nm, tag=nm)
                    tt[nm] = (t, t[:].rearrange("p (q c) -> p q c", c=28))
                return tt

            def emit_seg(tiles, st):
                xt = xts[st % 2]
                nc.sync.dma_start(xt[0:112, :], xh_d[st])
                ps = ppool.tile([128, b_blocks * 128], f32)
                for b in range(b_blocks):
                    nc.tensor.matmul(ps[:, 128 * b:128 * b + 84],
                                     xt[:, 128 * b:128 * b + 128],
                                     wp[:], start=True, stop=True)
                psv = ps[:].rearrange("p (b c) -> p b c", c=128)
                seg = emit_seg.idx
                emit_seg.idx += 1
                ssl = slice(seg * b_blocks, (seg + 1) * b_blocks)
                hl3v = tiles["hl3"][1]
                k3v = tiles["k3"][1]
                # PSUM evacuation (one PSUM operand per op); k3/f3 first so
                # DVE unblocks after a single ACT copy:
                #   k3  <- sc3 + M16 (fp16 write rounds to integer+M16)
                #   f3  <- (k3 - M16) - sc3[psum]
                #   hl3 <- HL cols;  g3 <- hl3 * HR[psum]
                nc.scalar.activation(k3v[:, ssl, :], psv[:, :, 56:84],
                                     Ident, bias=M16, scale=1.0)
                nc.vector.scalar_tensor_tensor(
                    tiles["f3"][1][:, ssl, :], k3v[:, ssl, :], M16,
                    psv[:, :, 56:84], sub, sub)
                nc.scalar.activation(hl3v[:, ssl, :], psv[:, :, 0:28],
                                     Ident, bias=0.0, scale=1.0)
                nc.vector.tensor_tensor(tiles["g3"][1][:, ssl, :],
                                        hl3v[:, ssl, :],
                                        psv[:, :, 28:56], mult)

            def wide_chunks(tiles, glen, st0):
                """Yield the wide-phase as a list of closures (chunks)."""
                q = glen * b_blocks
                qsl = slice(0, q)
                qf3, qf2, qf1 = q * 28, q * 14, q * 7
                g3 = tiles["g3"][0]
                f3 = tiles["f3"][0]

                def gt(cols, nm, dt=f16):
                    t = g1pool.tile([128, group * b_blocks * cols], dt,
                                    name=nm, tag=nm)
                    return t, t[:].rearrange("p (q c) -> p q c", c=cols)

                st_ = {}

                def c_t3u3():
                    t3, t3v = gt(28, "t3")
                    nc.scalar.activation(t3[:, 0:qf3], f3[:, 0:qf3], Sin,
                                         bias=0.0, scale=float(F32(-TWO_PI)))
                    u3, u3v = gt(28, "u3")
                    nc.vector.tensor_tensor(u3v[:, qsl, :], t3v[:, qsl, :],
                                            bc(0, 28, q), mult)
                    st_["u3"] = u3

                def c_o3():
                    o3, o3v = gt(28, "o3")
                    nc.vector.tensor_tensor(o3[:, 0:qf3], g3[:, 0:qf3],
                                            st_["u3"][:, 0:qf3], addop)
                    st_["o3"] = (o3, o3v)

                def c_s2():
                    # stored o3 is scaled exactly 1/2pi: s2 IS the phase in
                    # periods; round directly (per-node dfrac in the magic)
                    o3, o3v = st_["o3"]
                    st_["l2v"] = o3v[:, qsl, 0:14]
                    st_["r2v"] = o3v[:, qsl, 14:28]
                    s2, s2v = gt(14, "s2")
                    nc.vector.tensor_tensor(s2v[:, qsl, :], st_["l2v"],
                                            st_["r2v"], addop)
                    st_["s2"] = (s2, s2v)

                def c_k2():
                    s2, s2v = st_["s2"]
                    k2, k2v = gt(14, "k2")
                    for m in range(2):
                        d = n2[m]
                        nc.vector.tensor_scalar_add(
                            k2v[:, qsl, 7 * m:7 * m + 7],
                            s2v[:, qsl, 7 * m:7 * m + 7],
                            float(F32(M16 + d["dfrac"])))
                    nc.vector.tensor_scalar_sub(k2[:, 0:qf2], k2[:, 0:qf2],
                                                M16)
                    st_["k2"] = k2

                def c_f2():
                    s2, s2v = st_["s2"]
                    f2, f2v = gt(14, "f2")
                    nc.vector.tensor_tensor(f2[:, 0:qf2],
                                            st_["k2"][:, 0:qf2],
                                            s2[:, 0:qf2], sub)
                    t2, t2v = gt(14, "t2")
                    for m in range(2):
                        d = n2[m]
                        nc.scalar.activation(
                            t2v[:, qsl, 7 * m:7 * m + 7],
                            f2v[:, qsl, 7 * m:7 * m + 7], Sin,
                            bias=float(F32(TWO_PI * d["dfrac"])),
                            scale=float(F32(-TWO_PI)))
                    u2, u2v = gt(14, "u2")
                    nc.vector.tensor_tensor(u2v[:, qsl, :], t2v[:, qsl, :],
                                            bc(28, 42, q), mult)
                    st_["u2"] = u2

                def c_HL2():
                    HL2, HL2v = gt(14, "HL2")
                    nc.vector.tensor_tensor(HL2v[:, qsl, :], st_["l2v"],
                                            bc(56, 70, q), mult)
                    for m in range(2):
                        d = n2[m]
                        sl7 = (slice(None), qsl, slice(7 * m, 7 * m + 7))
                        nc.vector.tensor_scalar_add(HL2v[sl7], HL2v[sl7],
                                                    float(F32(d["hl_b"])))
                    st_["HL2"] = HL2

                def c_HR2():
                    HR2, HR2v = gt(14, "HR2")
                    for m in range(2):
                        d = n2[m]
                        sl7 = (slice(None), qsl, slice(7 * m, 7 * m + 7))
                        nc.vector.tensor_scalar_add(
                            HR2v[sl7], st_["r2v"][:, :, 7 * m:7 * m + 7],
                            float(F32(d["hr_b"])))
                    g2t, _ = gt(14, "g2")
                    nc.vector.tensor_tensor(g2t[:, 0:qf2],
                                            st_["HL2"][:, 0:qf2],
                                            HR2[:, 0:qf2], mult)
                    st_["g2"] = g2t

                def c_o2():
                    o2, o2v = gt(14, "o2")
                    nc.vector.tensor_tensor(o2[:, 0:qf2], st_["g2"][:, 0:qf2],
                                            st_["u2"][:, 0:qf2], addop)
                    st_["o2"] = (o2, o2v)

                def c_s1():
                    o2, o2v = st_["o2"]
                    st_["l1v"] = o2v[:, qsl, 0:7]
                    st_["r1v"] = o2v[:, qsl, 7:14]
                    d = n1
                    s1, s1v = gt(7, "s1")
                    nc.vector.tensor_tensor(s1v[:, qsl, :], st_["l1v"],
                                            st_["r1v"], addop)
                    # sc_sc == 1 (stored scale is 1/2pi): the f32 convert
                    # and dfrac shift merge into one ts_add
                    sc1, _ = gt(7, "sc1", dt=f32)
                    nc.vector.tensor_scalar_add(sc1[:, 0:qf1], s1[:, 0:qf1],
                                                float(F32(d["dfrac"])))
                    st_["sc1"] = sc1

                def c_f1():
                    d = n1
                    sc1 = st_["sc1"]
                    k1, _ = gt(7, "k1", dt=f32)
                    nc.vector.tensor_scalar_add(k1[:, 0:qf1], sc1[:, 0:qf1],
                                                M32)
                    f1, _ = gt(7, "f1")
                    nc.vector.scalar_tensor_tensor(
                        f1[:, 0:qf1], k1[:, 0:qf1], M32, sc1[:, 0:qf1],
                        sub, sub)
                    t1, t1v = gt(7, "t1")
                    nc.scalar.activation(t1[:, 0:qf1], f1[:, 0:qf1], Sin,
                                         bias=0.0, scale=float(F32(-TWO_PI)))
                    st_["t1"] = (t1, t1v)

                def c_g1():
                    d = n1
                    HL1, HL1v = gt(7, "HL1")
                    nc.vector.tensor_scalar_mul(HL1v[:, qsl, :], st_["l1v"],
                                                float(F32(d["hl_sc"])))
                    nc.vector.tensor_scalar_add(HL1[:, 0:qf1], HL1[:, 0:qf1],
                                                float(F32(d["hl_b"])))
                    HR1, HR1v = gt(7, "HR1")
                    nc.vector.tensor_scalar_add(HR1v[:, qsl, :], st_["r1v"],
                                                float(F32(d["hr_b"])))
                    g1t, _ = gt(7, "g1")
                    nc.vector.tensor_tensor(g1t[:, 0:qf1], HL1[:, 0:qf1],
                                            HR1[:, 0:qf1], mult)
                    st_["g1"] = g1t

                def c_yo():
                    # L1 has a single node: R is one scalar -> fused stt
                    t1, t1v = st_["t1"]
                    yo, _ = gt(7, "yo")
                    nc.vector.scalar_tensor_tensor(
                        yo[:, 0:qf1], t1[:, 0:qf1],
                        float(F32(n1["R"] * n1["c"])), st_["g1"][:, 0:qf1],
                        mult, addop)
                    dst = out_d[st0:st0 + glen].transpose([1, 0, 2, 3])
                    yo4 = yo[:, 0:qf1].rearrange("p (g b a) -> p g b a",
                                                 g=glen, a=SLOTS)
                    nc.sync.dma_start(dst, yo4)

                return [c_t3u3, c_o3, c_s2, c_k2, c_f2, c_HL2, c_HR2,
                        c_o2, c_s1, c_f1, c_g1, c_yo]

            # group sizes: small first group shortens the pipeline fill
            # ramp; small last group shortens the un-overlapped tail
            glens = []
            rem = n_st
            if n_st > group + 5:
                glens.append(2)
                rem -= 2
                tail = 3 if rem % group == 1 else (rem % group) or 3
                while rem > tail:
                    glens.append(min(group, rem - tail))
                    rem -= glens[-1]
                glens.append(rem)
                rem = 0
            while rem > 0:
                glens.append(min(group, rem))
                rem -= glens[-1]

            # software pipeline: interleave group g's wide chunks with
            # group g+1's seg ops so the (in-order) engines always have
            # ready work queued.
            prev = None
            st0 = 0
            for glen in glens:
                tiles = alloc_group()
                emit_seg.idx = 0
                segs = list(range(st0, st0 + glen))
                if prev is None:
                    for st in segs:
                        emit_seg(tiles, st)
                else:
                    # chunk-first: the first chunk (t3/u3) depends only on
                    # the previous group and keeps ACT/DVE busy while the
                    # new group's DMAs/matmuls start.
                    chunks = wide_chunks(*prev)
                    si = ci = 0
                    while ci < len(chunks) or si < len(segs):
                        if ci < len(chunks):
                            chunks[ci]()
                            ci += 1
                        if si < len(segs) and (ci * len(segs)
                                               >= si * len(chunks)):
                            emit_seg(tiles, segs[si])
                            si += 1
                prev = (tiles, glen, st0)
                st0 += glen
            for c in wide_chunks(*prev):
                c()

    nc.compile()
    _PROGRAM_CACHE[key] = nc
    return nc


def kernel(x, leaf_w, leaf_b, w1, b1, om1, w2, b2, om2, w3, b3, om3):
    from concourse.bass_interp import get_hw_module
    from concourse.bass_utils import run_bass_kernel_spmd

    x = np.ascontiguousarray(x, dtype=F32)
    wp, cst, consts = _fold(
        leaf_w, leaf_b, w1, b1, om1, w2, b2, om2, w3, b3, om3,
        x[:: max(1, N_FULL // 4096)][:4096])
    _build_program.consts = consts
    nc = _build_program()

    in_maps = []
    for c in range(N_CORES):
        xh = _pack_x(x[c * N_CORE:(c + 1) * N_CORE])
        in_maps.append({"xh": xh, "wp": wp, "cst": cst,
                        "ones": np.ones((16, B * 128), F16)})

    kw = {}
    if os.environ.get("KERNEL_TRACE_DIR"):
        kw["tmpdir"] = os.environ["KERNEL_TRACE_DIR"]
    old = nc.m
    nc.m = get_hw_module(nc.m)
    try:
        res = run_bass_kernel_spmd(nc, in_maps, core_ids=list(range(N_CORES)),
                                   **kw)
    finally:
        nc.m = old
    kernel._last = res

    S, T = consts["S"], consts["T"]
    out = np.empty(N_FULL, F32)
    for c in range(N_CORES):
        oc = res.results[c]["out"]          # [N_ST, 128, B, 7] f16
        oc = oc.transpose(0, 2, 1, 3).reshape(-1)[:N_CORE].astype(F32)
        out[c * N_CORE:(c + 1) * N_CORE] = F32(S) * oc + F32(T)
    return out
